# revision 1
# baseline (speedup 1.0000x reference)
"""BiMamba forward kernel for 8 TRN2 NeuronCores.

Sharding: core c = (batch b, direction dir, d_inner half h); the host
pre-flips reverse-direction inputs in time so the device program is
identical (purely causal) on all cores. Each core produces a partial
output projection [d_model, L]; the host sums four partials per batch
element (unflipping the reverse ones). A host-side channel permutation
puts this core's d_inner half in x-path tiles 0..5 so the single SPMD
program needs no per-core branches.

Device layout: channels on partitions, time on the free dim. The scan is
hardware tensor_tensor_scan (h = dA*h + dBu along time), one instruction
per (128-channel tile, state s). Decay planes: ScalarE exp for s<8 (fp32
for s<4, bf16 above); s>=8 as bf16 products e_s = e_8*e_{s-8} (fast
decay => bf16 rounding harmless). dBu/ws multiplies are bf16
tensor_tensor split between VectorE and GpSimd; sum_s + Dp-skip
accumulate in PSUM via identity/diagonal matmuls on TensorE; the causal
conv also runs on TensorE as 4 diagonal matmuls over shifted views.
B/C rows bounce through DRAM and return as partition-broadcast DMA
reads forming [128, L] replicated tiles.
"""
import numpy as np
import ml_dtypes

import concourse.bass as bass
import concourse.tile as tile
from concourse import bacc, mybir
from concourse.bass_utils import run_bass_kernel_spmd

D_MODEL = 768
D_INNER = 1536
D_STATE = 16
D_CONV = 4
DT_RANK = 48
BATCH = 2
SEQLEN = 2048

HALF = D_INNER // 2
NDT = HALF // 128            # 6 half d-tiles
NDT_FULL = D_INNER // 128    # 12 full d-tiles
NK = D_MODEL // 128          # 6 k-tiles over d_model
L = SEQLEN
NCH = 4
CW = L // NCH                # 512
NXD = DT_RANK + 2 * D_STATE  # 80
NXP = 96                     # x_dbl psum rows padded: B/C at partition 64
NM = D_MODEL // 128          # 6 out-proj row tiles

F32 = mybir.dt.float32
BF16 = mybir.dt.bfloat16
BF_NP = ml_dtypes.bfloat16

N_S_F32 = 4                                         # fp32 decay planes
USE_CC = True    # pairwise x_dbl via all-8 AllReduce (masked regions)
POOL_DBU = frozenset({0, 2, 4, 6, 8, 9, 10, 11, 12, 13, 14, 15})
POOL_WS = frozenset({0, 4, 8, 12, 14})

AF = mybir.ActivationFunctionType
OP = mybir.AluOpType


def build_program(debug_stage=0):
    nc = bacc.Bacc("TRN2", target_bir_lowering=False, debug=False,
                   num_devices=8)
    dram = {}

    def din(name, shape, dt):
        dram[name] = nc.dram_tensor(name, list(shape), dt,
                                    kind="ExternalInput").ap()

    def dout(name, shape, dt):
        dram[name] = nc.dram_tensor(name, list(shape), dt,
                                    kind="ExternalOutput").ap()

    din("uT", (D_MODEL, L), BF16)
    din("w_in_xT", (D_MODEL, D_INNER), BF16)
    din("w_in_zT", (D_MODEL, HALF), BF16)
    din("conv_diag", (NDT_FULL * D_CONV * 128, 128), BF16)
    din("conv_b", (D_INNER, 1), F32)
    din("w_xT", (D_INNER, NXP), BF16)
    din("w_dtT", (DT_RANK, HALF), BF16)
    din("b_dt", (HALF, 1), F32)
    din("A_half", (HALF, D_STATE), F32)
    din("dp_diag", (NDT * 128, 128), BF16)
    din("idn", (128, 128), BF16)
    din("w_outT", (HALF, D_MODEL), BF16)
    din("mask_cols", (NXP, 4), F32)

    if debug_stage == 1:
        dout("xc_dbg", (HALF if USE_CC else D_INNER, L), F32)
        dout("delta_dbg", (HALF, L), F32)
        dout("xdbl_dbg", (NXP, L), F32)
    dout("out_part", (D_MODEL, L), F32)

    with tile.TileContext(nc) as tc:
        _body_once(nc, tc, dram, debug_stage)
    nc.compile()
    return nc


def _body_once(nc, tc, dram, dbg):
    with tc.tile_pool(name="wpool", bufs=1) as wp, \
         tc.tile_pool(name="dramp", bufs=1, space="DRAM") as dp_pool:

        # ---- DRAM scratch (tracked tiles) ----
        bc_scr = dp_pool.tile([2 * D_STATE, L], BF16, name="bc_scr")
        cc_in = dp_pool.tile([NXP, 4 * L], F32, name="cc_in")
        cc_out = dp_pool.tile([NXP, 4 * L], F32, name="cc_out")
        z_scr = [dp_pool.tile([128, L], BF16, name=f"z_scr{r}")
                 for r in range(NDT)]
        xc_scr = [dp_pool.tile([128, L], BF16, name=f"xc_scr{r}")
                  for r in range(NDT)]

        # ---- persistent small weights ----
        idn = wp.tile([128, 128], BF16, name="idn")
        nc.sync.dma_start(idn[:], dram["idn"][:])
        dp_diag = [wp.tile([128, 128], BF16, name=f"dpd{r}")
                   for r in range(NDT)]
        A_col = [wp.tile([128, D_STATE], F32, name=f"acol{r}")
                 for r in range(NDT)]
        b_dt = [wp.tile([128, 1], F32, name=f"bdt{r}") for r in range(NDT)]
        conv_b = [wp.tile([128, 1], F32, name=f"cvb{r}")
                  for r in range(NDT_FULL)]
        for r in range(NDT):
            nc.sync.dma_start(dp_diag[r][:],
                              dram["dp_diag"][r * 128:(r + 1) * 128, :])
            nc.sync.dma_start(A_col[r][:],
                              dram["A_half"][r * 128:(r + 1) * 128, :])
            nc.sync.dma_start(b_dt[r][:],
                              dram["b_dt"][r * 128:(r + 1) * 128, :])
        for r in range(NDT_FULL):
            nc.sync.dma_start(conv_b[r][:],
                              dram["conv_b"][r * 128:(r + 1) * 128, :])
        w_dtT = wp.tile([DT_RANK, HALF], BF16, name="w_dtT")
        nc.sync.dma_start(w_dtT[:], dram["w_dtT"][:])
        w_outT = [wp.tile([128, D_MODEL], BF16, name=f"wout{r}")
                  for r in range(NDT)]
        for r in range(NDT):
            nc.sync.dma_start(w_outT[r][:],
                              dram["w_outT"][r * 128:(r + 1) * 128, :])
        NDT_X = NDT if USE_CC else NDT_FULL
        w_xT = [wp.tile([128, NXP], BF16, name=f"wx{k}")
                for k in range(NDT_X)]
        for k in range(NDT_X):
            nc.sync.dma_start(w_xT[k][:],
                              dram["w_xT"][k * 128:(k + 1) * 128, :])
        mask_cols = wp.tile([NXP, 4], F32, name="mask_cols")
        nc.sync.dma_start(mask_cols[:], dram["mask_cols"][:])

        with tc.tile_pool(name="hold", bufs=1) as hold:
            dtT_bf = hold.tile([DT_RANK, L], BF16, name="dtT_bf")
            yg_bf = [hold.tile([128, L], BF16, name=f"yg{r}")
                     for r in range(NDT)]

            _stages_123(nc, tc, dram, dbg, wp, locals())
            _scan_stage(nc, tc, dram, dbg, wp, locals())

            # ---------- stage 6: out-proj ----------
            with tc.tile_pool(name="op6", bufs=1) as p6, \
                 tc.tile_pool(name="ps6", bufs=2, space="PSUM") as ps6:
                for m in range(NM):
                    for n in range(NCH):
                        ps = ps6.tile([128, CW], F32, name="ps6t",
                                      tag="ps6t")
                        for r in range(NDT):
                            nc.tensor.matmul(
                                ps[:], w_outT[r][:, m * 128:(m + 1) * 128],
                                yg_bf[r][:, n * CW:(n + 1) * CW],
                                start=(r == 0), stop=(r == NDT - 1))
                        ot = p6.tile([128, CW], F32, name="ot", tag="ot",
                                     bufs=3)
                        nc.scalar.copy(ot[:], ps[:])
                        nc.sync.dma_start(
                            dram["out_part"][m * 128:(m + 1) * 128,
                                             n * CW:(n + 1) * CW], ot[:])


def _stages_123(nc, tc, dram, dbg, wp, env):
    hold = env["hold"]
    dtT_bf = env["dtT_bf"]
    conv_b = env["conv_b"]
    w_xT = env["w_xT"]
    bc_scr = env["bc_scr"]
    z_scr = env["z_scr"]
    xc_scr = env["xc_scr"]
    mask_cols = env["mask_cols"]
    cc_in = env["cc_in"]
    cc_out = env["cc_out"]
    LPAD = L + 3
    NDT_X = NDT if USE_CC else NDT_FULL

    with tc.tile_pool(name="pre3", bufs=1) as p3, \
         tc.tile_pool(name="ps_a", bufs=2, space="PSUM") as psa:
        xc_bf = [p3.tile([128, L], BF16, name=f"xc{r}", tag=f"xc{r}")
                 for r in range(NDT_X)]
        uT = [p3.tile([128, L], BF16, name=f"uT{k}", tag=f"uT{k}")
              for k in range(NK)]
        for k in range(NK):
            nc.sync.dma_start(uT[k][:],
                              dram["uT"][k * 128:(k + 1) * 128, :])
        w_in_zT = [p3.tile([128, HALF], BF16, name=f"wiz{k}",
                           tag=f"wiz{k}") for k in range(NK)]
        for k in range(NK):
            nc.sync.dma_start(w_in_zT[k][:],
                              dram["w_in_zT"][k * 128:(k + 1) * 128, :])
        with tc.tile_pool(name="pre12", bufs=1) as p12:
            WIX_W = NDT_X * 128
            w_in_xT = [p12.tile([128, WIX_W], BF16, name=f"wix{k}",
                                tag=f"wix{k}") for k in range(NK)]
            for k in range(NK):
                nc.sync.dma_start(
                    w_in_xT[k][:],
                    dram["w_in_xT"][k * 128:(k + 1) * 128, 0:WIX_W])
            conv_diag = [p12.tile([128, 128], BF16, name=f"cvd{i}",
                                  tag=f"cvd{i}")
                         for i in range(NDT_X * D_CONV)]
            for i in range(NDT_X * D_CONV):
                nc.sync.dma_start(conv_diag[i][:],
                                  dram["conv_diag"][i * 128:(i + 1) * 128, :])

            # ---- stages 1+2 fused per d-tile: in-proj -> conv -> silu ----
            for r in range(NDT_X):
                xr = p12.tile([128, LPAD], BF16, name="xr", tag="xr",
                              bufs=2)
                nc.vector.memset(xr[:, 0:3], 0.0)
                for n in range(NCH):
                    ps = psa.tile([128, CW], F32, name="psa", tag="psa")
                    for k in range(NK):
                        nc.tensor.matmul(
                            ps[:], w_in_xT[k][:, r * 128:(r + 1) * 128],
                            uT[k][:, n * CW:(n + 1) * CW],
                            start=(k == 0), stop=(k == NK - 1))
                    nc.vector.tensor_copy(
                        xr[:, 3 + n * CW:3 + (n + 1) * CW], ps[:])
                for n in range(NCH):
                    ps = psa.tile([128, CW], F32, name="psa", tag="psa")
                    for j in range(D_CONV):
                        nc.tensor.matmul(
                            ps[:], conv_diag[r * D_CONV + j][:],
                            xr[:, n * CW + j:n * CW + j + CW],
                            start=(j == 0), stop=(j == D_CONV - 1))
                    nc.scalar.activation(xc_bf[r][:, n * CW:(n + 1) * CW],
                                         ps[:], AF.Silu,
                                         bias=conv_b[r][:], scale=1.0)
            for r in range(NDT):
                nc.sync.dma_start(xc_scr[r][:], xc_bf[r][:])

        # ---- stage 3: x_dbl (partial if USE_CC, then AllReduce) ----
        xdbl_f = p3.tile([NXP, L], F32, name="xdbl_f", tag="xdbl_f")
        for n in range(NCH):
            ps = psa.tile([NXP, CW], F32, name="ps3", tag="ps3")
            for k in range(NDT_X):
                nc.tensor.matmul(ps[:], w_xT[k][:],
                                 xc_bf[k][:, n * CW:(n + 1) * CW],
                                 start=(k == 0), stop=(k == NDT_X - 1))
            nc.vector.tensor_copy(xdbl_f[:, n * CW:(n + 1) * CW], ps[:])

        if USE_CC:
            # masked region writes -> all-8 AllReduce == pairwise sums
            for j in range(4):
                mreg = p3.tile([NXP, L], F32, name="mreg", tag="mreg",
                               bufs=2)
                nc.vector.tensor_scalar(
                    mreg[:], xdbl_f[:],
                    mask_cols[:, j:j + 1], None, OP.mult)
                nc.sync.dma_start(cc_in[:, j * L:(j + 1) * L], mreg[:])
            nc.gpsimd.collective_compute(
                "AllReduce", mybir.AluOpType.add,
                replica_groups=[[0, 1, 2, 3, 4, 5, 6, 7]],
                ins=[cc_in[:]], outs=[cc_out[:]])
            # masked-select own pair region back
            xsel = p3.tile([NXP, L], F32, name="xsel", tag="xsel")
            for j in range(4):
                rreg = p3.tile([NXP, L], F32, name="rreg", tag="rreg",
                               bufs=2)
                nc.sync.dma_start(rreg[:],
                                  cc_out[:, j * L:(j + 1) * L])
                if j == 0:
                    nc.vector.tensor_scalar(
                        xsel[:], rreg[:],
                        mask_cols[:, 0:1], None, OP.mult)
                else:
                    nc.vector.scalar_tensor_tensor(
                        xsel[:], rreg[:],
                        mask_cols[:, j:j + 1], xsel[:],
                        OP.mult, OP.add)
            xdbl_use = xsel
        else:
            xdbl_use = xdbl_f

        nc.vector.tensor_copy(dtT_bf[:], xdbl_use[0:DT_RANK, :])
        bcb = p3.tile([2 * D_STATE, L], BF16, name="bcb", tag="bcb")
        nc.vector.tensor_copy(bcb[:], xdbl_use[64:NXP, :])
        nc.sync.dma_start(bc_scr[:], bcb[:])
        if dbg == 1:
            nc.sync.dma_start(dram["xdbl_dbg"][:], xdbl_use[:])

        # z half -> silu -> spill gz (overlaps scan start on PE/ACT)
        for r in range(NDT):
            zt = p3.tile([128, L], BF16, name="zt", tag="zt", bufs=2)
            for n in range(NCH):
                ps = psa.tile([128, CW], F32, name="psz", tag="psz",
                              bufs=2)
                for k in range(NK):
                    nc.tensor.matmul(
                        ps[:], w_in_zT[k][:, r * 128:(r + 1) * 128],
                        uT[k][:, n * CW:(n + 1) * CW],
                        start=(k == 0), stop=(k == NK - 1))
                nc.vector.tensor_copy(zt[:, n * CW:(n + 1) * CW], ps[:])
            gzt = p3.tile([128, L], BF16, name="gzt", tag="gzt", bufs=2)
            nc.scalar.activation(gzt[:], zt[:], AF.Silu)
            nc.sync.dma_start(z_scr[r][:], gzt[:])

        if dbg == 1:
            for r in range(NDT_X):
                xcd = p3.tile([128, L], F32, name="xcd", tag="xcd", bufs=2)
                nc.vector.tensor_copy(xcd[:], xc_bf[r][:])
                nc.sync.dma_start(dram["xc_dbg"][r * 128:(r + 1) * 128, :],
                                  xcd[:])


def _scan_stage(nc, tc, dram, dbg, wp, env):
    dtT_bf = env["dtT_bf"]
    yg_bf = env["yg_bf"]
    bc_scr = env["bc_scr"]
    z_scr = env["z_scr"]
    xc_scr = env["xc_scr"]
    w_dtT = env["w_dtT"]
    A_col = env["A_col"]
    b_dt = env["b_dt"]
    dp_diag = env["dp_diag"]
    idn = env["idn"]

    with tc.tile_pool(name="scanp", bufs=1) as sp, \
         tc.tile_pool(name="ps_mm4", bufs=2, space="PSUM") as ps4, \
         tc.tile_pool(name="ps_y", bufs=1, space="PSUM") as psy:
        for r in range(NDT):
            # ---- delta = softplus(dt @ W_dt.T + b_dt) ----
            zb = sp.tile([128, L], F32, name="zb", tag="zb")
            for n in range(NCH):
                ps = ps4.tile([128, CW], F32, name="ps4t", tag="ps4t")
                nc.tensor.matmul(ps[:], w_dtT[:, r * 128:(r + 1) * 128],
                                 dtT_bf[:, n * CW:(n + 1) * CW],
                                 start=True, stop=True)
                nc.vector.tensor_scalar(zb[:, n * CW:(n + 1) * CW], ps[:],
                                        b_dt[r][:], None, OP.add)
            tA = sp.tile([128, L], F32, name="tA", tag="tA")
            nc.scalar.activation(tA[:], zb[:], AF.Abs)       # |z|
            tB = sp.tile([128, L], F32, name="tB", tag="tB")
            nc.scalar.activation(tB[:], tA[:], AF.Exp, bias=0.0,
                                 scale=-1.0)                 # exp(-|z|)
            tL = sp.tile([128, L], F32, name="tL", tag="tA")
            nc.scalar.activation(tL[:], tB[:], AF.Ln, bias=1.0,
                                 scale=1.0)                  # ln(1+e)
            zmx = sp.tile([128, L], F32, name="zmx", tag="tB")
            nc.vector.tensor_scalar(zmx[:], zb[:], 0.0, None, OP.max)
            delta = sp.tile([128, L], F32, name="delta", tag="delta",
                            bufs=2)
            nc.vector.tensor_tensor(delta[:], zmx[:], tL[:], OP.add)
            if dbg == 1:
                nc.sync.dma_start(
                    dram["delta_dbg"][r * 128:(r + 1) * 128, :], delta[:])

            # ---- du = delta * xc ----
            xcr = sp.tile([128, L], BF16, name="xcr", tag="xcr", bufs=2)
            nc.sync.dma_start(xcr[:], xc_scr[r][:])
            du = sp.tile([128, L], BF16, name="du", tag="du", bufs=2)
            nc.vector.tensor_tensor(du[:], delta[:], xcr[:], OP.mult)

            yp = [psy.tile([128, CW], F32, name=f"yp{n}", tag=f"yp{n}")
                  for n in range(NCH)]

            for s in range(D_STATE):
                # dA plane straight from ScalarE
                if s < N_S_F32:
                    dA = sp.tile([128, L], F32, name="ef", tag="ef", bufs=3)
                else:
                    dA = sp.tile([128, L], BF16, name="eb", tag="eb",
                                 bufs=3)
                nc.scalar.activation(dA[:], delta[:], AF.Exp, bias=0.0,
                                     scale=A_col[r][:, s:s + 1])
                b_rep = sp.tile([128, L], BF16, name="b_rep", tag="b_rep",
                                bufs=3)
                nc.sync.dma_start(
                    b_rep[:], bc_scr[s:s + 1, :].broadcast_to((128, L)))
                c_rep = sp.tile([128, L], BF16, name="c_rep", tag="c_rep",
                                bufs=3)
                nc.sync.dma_start(
                    c_rep[:], bc_scr[D_STATE + s:D_STATE + s + 1, :]
                    .broadcast_to((128, L)))
                dbu = sp.tile([128, L], BF16, name="dbu", tag="dbu",
                              bufs=3)
                eng = nc.gpsimd if s in POOL_DBU else nc.vector
                eng.tensor_tensor(dbu[:], du[:], b_rep[:], OP.mult)
                h = sp.tile([128, L], BF16, name="h", tag="h", bufs=3)
                nc.vector.tensor_tensor_scan(h[:], dA[:], dbu[:], 0.0,
                                             OP.mult, OP.add)
                ws = sp.tile([128, L], BF16, name="ws", tag="ws", bufs=2)
                eng2 = nc.gpsimd if s in POOL_WS else nc.vector
                eng2.tensor_tensor(ws[:], h[:], c_rep[:], OP.mult)
                for n in range(NCH):
                    nc.tensor.matmul(yp[n][:], idn[:],
                                     ws[:, n * CW:(n + 1) * CW],
                                     start=(s == 0), stop=False)
            # skip term
            for n in range(NCH):
                nc.tensor.matmul(yp[n][:], dp_diag[r][:],
                                 xcr[:, n * CW:(n + 1) * CW],
                                 start=False, stop=True)
            # gate with silu(z) (precomputed gz)
            gz = sp.tile([128, L], BF16, name="gz", tag="gz", bufs=2)
            nc.sync.dma_start(gz[:], z_scr[r][:])
            for n in range(NCH):
                nc.vector.tensor_tensor(yg_bf[r][:, n * CW:(n + 1) * CW],
                                        yp[n][:],
                                        gz[:, n * CW:(n + 1) * CW],
                                        OP.mult)


# ======================= host side =======================

def _prep_core_inputs(inputs, b, rev, h):
    hs = np.asarray(inputs["hidden_states"])
    W_in = np.asarray(inputs["W_in"])
    conv_w = np.asarray(inputs["conv_w"])[:, 0, :]
    conv_b = np.asarray(inputs["conv_b"])
    W_x = np.asarray(inputs["W_x"])
    W_dt = np.asarray(inputs["W_dt"])
    b_dt = np.asarray(inputs["b_dt"])
    A = -np.exp(np.asarray(inputs["A_log"], np.float64)).astype(np.float32)
    Dp = np.asarray(inputs["Dp"])
    W_out = np.asarray(inputs["W_out"])

    lo, hi = h * HALF, (h + 1) * HALF
    perm = np.r_[lo:hi, (0 if h else HALF):(HALF if h else D_INNER)]

    u = hs[b]
    if rev:
        u = u[::-1]
    uT = np.ascontiguousarray(u.T).astype(BF_NP)

    W_in_x = W_in[0:D_INNER][perm]
    W_in_z = W_in[D_INNER + lo:D_INNER + hi]
    conv_wp = conv_w[perm]
    conv_bp = conv_b[perm].reshape(-1, 1).astype(np.float32)
    W_xp = W_x[:, perm]
    W_xpad = np.zeros((NXP, W_xp.shape[1]), W_xp.dtype)
    W_xpad[0:DT_RANK] = W_xp[0:DT_RANK]
    W_xpad[64:96] = W_xp[DT_RANK:NXD]

    conv_diag = np.zeros((NDT_FULL * D_CONV * 128, 128), np.float32)
    idx = np.arange(128)
    for r in range(NDT_FULL):
        for j in range(D_CONV):
            base = (r * D_CONV + j) * 128
            conv_diag[base + idx, idx] = conv_wp[r * 128:(r + 1) * 128, j]

    dp_diag = np.zeros((NDT * 128, 128), np.float32)
    for r in range(NDT):
        dp_diag[r * 128 + idx, idx] = Dp[lo + r * 128: lo + (r + 1) * 128]

    pair = (b << 1) | rev
    mask_cols = np.zeros((NXP, 4), np.float32)
    mask_cols[:, pair] = 1.0

    return {
        "uT": uT,
        "w_in_xT": np.ascontiguousarray(W_in_x.T).astype(BF_NP),
        "w_in_zT": np.ascontiguousarray(W_in_z.T).astype(BF_NP),
        "conv_diag": conv_diag.astype(BF_NP),
        "conv_b": conv_bp,
        "w_xT": np.ascontiguousarray(W_xpad.T).astype(BF_NP),
        "w_dtT": np.ascontiguousarray(W_dt[lo:hi].T).astype(BF_NP),
        "b_dt": b_dt[lo:hi].reshape(-1, 1).astype(np.float32),
        "A_half": A[lo:hi].astype(np.float32),
        "dp_diag": dp_diag.astype(BF_NP),
        "idn": np.eye(128, dtype=np.float32).astype(BF_NP),
        "w_outT": np.ascontiguousarray(W_out[:, lo:hi].T).astype(BF_NP),
        "mask_cols": mask_cols,
    }


_CACHE = {}


def kernel(**inputs):
    if "prog" not in _CACHE:
        _CACHE["prog"] = build_program(0)
    nc = _CACHE["prog"]

    in_maps = []
    for c in range(8):
        b, rev, h = c >> 2, (c >> 1) & 1, c & 1
        in_maps.append(_prep_core_inputs(inputs, b, rev, h))
    res = run_bass_kernel_spmd(nc, in_maps, list(range(8)))

    out = np.zeros((BATCH, L, D_MODEL), np.float32)
    for c in range(8):
        b, rev, h = c >> 2, (c >> 1) & 1, c & 1
        part = res.results[c]["out_part"].T
        if rev:
            part = part[::-1]
        out[b] += part
    return out



# revision 6
# speedup vs baseline: 1.1915x; 1.1915x over previous
"""BiMamba forward kernel for 8 TRN2 NeuronCores.

Sharding: core c = (batch b, direction dir, d_inner half h); the host
pre-flips reverse-direction inputs in time so the device program is
identical (purely causal) on all cores. Each core produces a partial
output projection [d_model, L]; the host sums four partials per batch
element (unflipping the reverse ones). A host-side channel permutation
puts this core's d_inner half in x-path tiles 0..5 so the single SPMD
program needs no per-core branches.

Device layout: channels on partitions, time on the free dim. Each core
computes the in-proj/conv for the FULL d_inner locally (PE has slack)
so x_dbl needs no collective. The scan is hardware tensor_tensor_scan
(h = dA*h + dBu along time), one instruction per (128-channel tile,
state s). All elementwise work stays on VectorE: running tensor_tensor
on GpSimd concurrently halves DVE throughput (shared SBUF ports), so
the Pool engine is left idle on purpose. Decay planes dA_s come from
ScalarE exp (per-partition scale = A[:,s]); delta is a single Softplus
activation straight from PSUM. sum_s + Dp-skip accumulate in PSUM via
identity/diagonal matmuls on TensorE; the causal conv also runs on
TensorE as 4 diagonal matmuls over shifted views. B/C rows bounce
through DRAM and return as partition-broadcast DMA reads forming
[128, L] replicated tiles.
"""
import numpy as np
import ml_dtypes

import concourse.bass as bass
import concourse.tile as tile
from concourse import bacc, mybir
from concourse.bass_utils import run_bass_kernel_spmd

D_MODEL = 768
D_INNER = 1536
D_STATE = 16
D_CONV = 4
DT_RANK = 48
BATCH = 2
SEQLEN = 2048

HALF = D_INNER // 2
NDT = HALF // 128            # 6 half d-tiles
NDT_FULL = D_INNER // 128    # 12 full d-tiles
NK = D_MODEL // 128          # 6 k-tiles over d_model
L = SEQLEN
NCH = 4
CW = L // NCH                # 512
NXD = DT_RANK + 2 * D_STATE  # 80
NXP = 96                     # x_dbl psum rows padded: B/C at partition 64
NM = D_MODEL // 128          # 6 out-proj row tiles

F32 = mybir.dt.float32
BF16 = mybir.dt.bfloat16
BF_NP = ml_dtypes.bfloat16

N_S_F32 = 4                  # fp32 decay planes for slow-decaying states

AF = mybir.ActivationFunctionType
OP = mybir.AluOpType


def build_program(debug_stage=0):
    nc = bacc.Bacc("TRN2", target_bir_lowering=False, debug=False,
                   num_devices=8)
    dram = {}

    def din(name, shape, dt):
        dram[name] = nc.dram_tensor(name, list(shape), dt,
                                    kind="ExternalInput").ap()

    def dout(name, shape, dt):
        dram[name] = nc.dram_tensor(name, list(shape), dt,
                                    kind="ExternalOutput").ap()

    din("uT", (D_MODEL, L), BF16)
    din("w_in_xT", (D_MODEL, D_INNER), BF16)
    din("w_in_zT", (D_MODEL, HALF), BF16)
    din("conv_diag", (NDT_FULL * D_CONV * 128, 128), BF16)
    din("conv_b", (D_INNER, 1), F32)
    din("w_xT", (D_INNER, NXP), BF16)
    din("w_dtT", (DT_RANK, HALF), BF16)
    din("b_dt", (HALF, 1), F32)
    din("A_half", (HALF, D_STATE), F32)
    din("dp_diag", (NDT * 128, 128), BF16)
    din("idn", (128, 128), BF16)
    din("w_outT", (HALF, D_MODEL), BF16)

    if debug_stage == 1:
        dout("xc_dbg", (D_INNER, L), F32)
        dout("delta_dbg", (HALF, L), F32)
        dout("xdbl_dbg", (NXP, L), F32)
    dout("out_part", (D_MODEL, L), F32)

    with tile.TileContext(nc) as tc:
        _body_once(nc, tc, dram, debug_stage)
    nc.compile()
    return nc


def _body_once(nc, tc, dram, dbg):
    with tc.tile_pool(name="wpool", bufs=1) as wp, \
         tc.tile_pool(name="dramp", bufs=1, space="DRAM") as dp_pool:

        # ---- DRAM scratch for B/C partition-broadcast bounce ----
        bc_scr = dp_pool.tile([2 * D_STATE, L], BF16, name="bc_scr")

        # ---- persistent small weights ----
        idn = wp.tile([128, 128], BF16, name="idn")
        nc.sync.dma_start(idn[:], dram["idn"][:])
        dp_diag = [wp.tile([128, 128], BF16, name=f"dpd{r}")
                   for r in range(NDT)]
        A_col = [wp.tile([128, D_STATE], F32, name=f"acol{r}")
                 for r in range(NDT)]
        b_dt = [wp.tile([128, 1], F32, name=f"bdt{r}") for r in range(NDT)]
        conv_b = [wp.tile([128, 1], F32, name=f"cvb{r}")
                  for r in range(NDT_FULL)]
        for r in range(NDT):
            nc.sync.dma_start(dp_diag[r][:],
                              dram["dp_diag"][r * 128:(r + 1) * 128, :])
            nc.sync.dma_start(A_col[r][:],
                              dram["A_half"][r * 128:(r + 1) * 128, :])
            nc.sync.dma_start(b_dt[r][:],
                              dram["b_dt"][r * 128:(r + 1) * 128, :])
        for r in range(NDT_FULL):
            nc.sync.dma_start(conv_b[r][:],
                              dram["conv_b"][r * 128:(r + 1) * 128, :])
        w_dtT = wp.tile([DT_RANK, HALF], BF16, name="w_dtT")
        nc.sync.dma_start(w_dtT[:], dram["w_dtT"][:])
        w_outT = [wp.tile([128, D_MODEL], BF16, name=f"wout{r}")
                  for r in range(NDT)]
        for r in range(NDT):
            nc.sync.dma_start(w_outT[r][:],
                              dram["w_outT"][r * 128:(r + 1) * 128, :])
        w_xT = [wp.tile([128, NXP], BF16, name=f"wx{k}")
                for k in range(NDT_FULL)]
        for k in range(NDT_FULL):
            nc.sync.dma_start(w_xT[k][:],
                              dram["w_xT"][k * 128:(k + 1) * 128, :])

        with tc.tile_pool(name="hold", bufs=1) as hold:
            xdbl_bf = hold.tile([NXP, L], BF16, name="xdbl_bf")
            yg_bf = [hold.tile([128, L], BF16, name=f"yg{r}")
                     for r in range(NDT)]
            xc_own = [hold.tile([128, L], BF16, name=f"xco{r}")
                      for r in range(NDT)]
            gz = [hold.tile([128, L], BF16, name=f"gz{r}")
                  for r in range(NDT)]

            env = {"hold": hold, "xdbl_bf": xdbl_bf, "yg_bf": yg_bf,
                   "xc_own": xc_own, "gz": gz, "bc_scr": bc_scr,
                   "conv_b": conv_b, "w_xT": w_xT, "w_dtT": w_dtT,
                   "A_col": A_col, "b_dt": b_dt, "dp_diag": dp_diag,
                   "idn": idn}
            _stages_123(nc, tc, dram, dbg, wp, env)
            _scan_stage(nc, tc, dram, dbg, wp, env)

            # ---------- stage 6: out-proj ----------
            with tc.tile_pool(name="op6", bufs=1) as p6, \
                 tc.tile_pool(name="ps6", bufs=2, space="PSUM") as ps6:
                for m in range(NM):
                    for n in range(NCH):
                        ps = ps6.tile([128, CW], F32, name="ps6t",
                                      tag="ps6t")
                        for r in range(NDT):
                            nc.tensor.matmul(
                                ps[:], w_outT[r][:, m * 128:(m + 1) * 128],
                                yg_bf[r][:, n * CW:(n + 1) * CW],
                                start=(r == 0), stop=(r == NDT - 1))
                        ot = p6.tile([128, CW], F32, name="ot", tag="ot",
                                     bufs=3)
                        nc.scalar.copy(ot[:], ps[:])
                        nc.sync.dma_start(
                            dram["out_part"][m * 128:(m + 1) * 128,
                                             n * CW:(n + 1) * CW], ot[:])


def _stages_123(nc, tc, dram, dbg, wp, env):
    xdbl_bf = env["xdbl_bf"]
    xc_own = env["xc_own"]
    gz = env["gz"]
    conv_b = env["conv_b"]
    w_xT = env["w_xT"]
    bc_scr = env["bc_scr"]
    LPAD = L + 3

    with tc.tile_pool(name="pre3", bufs=1) as p3, \
         tc.tile_pool(name="ps_a", bufs=2, space="PSUM") as psa:
        xc_oth = [p3.tile([128, L], BF16, name=f"xoth{r}", tag=f"xoth{r}")
                  for r in range(NDT_FULL - NDT)]
        uT = [p3.tile([128, L], BF16, name=f"uT{k}", tag=f"uT{k}")
              for k in range(NK)]
        for k in range(NK):
            nc.sync.dma_start(uT[k][:],
                              dram["uT"][k * 128:(k + 1) * 128, :])
        w_in_zT = [p3.tile([128, HALF], BF16, name=f"wiz{k}",
                           tag=f"wiz{k}") for k in range(NK)]
        for k in range(NK):
            nc.sync.dma_start(w_in_zT[k][:],
                              dram["w_in_zT"][k * 128:(k + 1) * 128, :])
        with tc.tile_pool(name="pre12", bufs=1) as p12:
            w_in_xT = [p12.tile([128, D_INNER], BF16, name=f"wix{k}",
                                tag=f"wix{k}") for k in range(NK)]
            for k in range(NK):
                nc.sync.dma_start(w_in_xT[k][:],
                                  dram["w_in_xT"][k * 128:(k + 1) * 128, :])
            conv_diag = [p12.tile([128, 128], BF16, name=f"cvd{i}",
                                  tag=f"cvd{i}")
                         for i in range(NDT_FULL * D_CONV)]
            for i in range(NDT_FULL * D_CONV):
                nc.sync.dma_start(conv_diag[i][:],
                                  dram["conv_diag"][i * 128:(i + 1) * 128, :])

            # ---- stages 1+2 fused per d-tile: in-proj -> conv -> silu ----
            for r in range(NDT_FULL):
                xr = p12.tile([128, LPAD], BF16, name="xr", tag="xr",
                              bufs=2)
                nc.vector.memset(xr[:, 0:3], 0.0)
                for n in range(NCH):
                    ps = psa.tile([128, CW], F32, name="psa", tag="psa")
                    for k in range(NK):
                        nc.tensor.matmul(
                            ps[:], w_in_xT[k][:, r * 128:(r + 1) * 128],
                            uT[k][:, n * CW:(n + 1) * CW],
                            start=(k == 0), stop=(k == NK - 1))
                    nc.vector.tensor_copy(
                        xr[:, 3 + n * CW:3 + (n + 1) * CW], ps[:])
                xc_dst = xc_own[r] if r < NDT else xc_oth[r - NDT]
                for n in range(NCH):
                    ps = psa.tile([128, CW], F32, name="psa", tag="psa")
                    for j in range(D_CONV):
                        nc.tensor.matmul(
                            ps[:], conv_diag[r * D_CONV + j][:],
                            xr[:, n * CW + j:n * CW + j + CW],
                            start=(j == 0), stop=(j == D_CONV - 1))
                    nc.scalar.activation(xc_dst[:, n * CW:(n + 1) * CW],
                                         ps[:], AF.Silu,
                                         bias=conv_b[r][:], scale=1.0)

        # ---- stage 3: x_dbl over the full d_inner (no collective) ----
        for n in range(NCH):
            ps = psa.tile([NXP, CW], F32, name="ps3", tag="ps3")
            for k in range(NDT_FULL):
                src = xc_own[k] if k < NDT else xc_oth[k - NDT]
                nc.tensor.matmul(ps[:], w_xT[k][:],
                                 src[:, n * CW:(n + 1) * CW],
                                 start=(k == 0), stop=(k == NDT_FULL - 1))
            nc.scalar.copy(xdbl_bf[:, n * CW:(n + 1) * CW], ps[:])

        nc.sync.dma_start(bc_scr[:], xdbl_bf[64:NXP, :])
        if dbg == 1:
            xdbg = p3.tile([NXP, L], F32, name="xdbg", tag="xdbg")
            nc.vector.tensor_copy(xdbg[:], xdbl_bf[:])
            nc.sync.dma_start(dram["xdbl_dbg"][:], xdbg[:])

        # ---- z half -> silu(z) straight from PSUM ----
        for r in range(NDT):
            for n in range(NCH):
                ps = psa.tile([128, CW], F32, name="psz", tag="psz",
                              bufs=2)
                for k in range(NK):
                    nc.tensor.matmul(
                        ps[:], w_in_zT[k][:, r * 128:(r + 1) * 128],
                        uT[k][:, n * CW:(n + 1) * CW],
                        start=(k == 0), stop=(k == NK - 1))
                nc.scalar.activation(gz[r][:, n * CW:(n + 1) * CW],
                                     ps[:], AF.Silu)

        if dbg == 1:
            for r in range(NDT_FULL):
                src = xc_own[r] if r < NDT else xc_oth[r - NDT]
                xcd = p3.tile([128, L], F32, name="xcd", tag="xcd", bufs=2)
                nc.vector.tensor_copy(xcd[:], src[:])
                nc.sync.dma_start(dram["xc_dbg"][r * 128:(r + 1) * 128, :],
                                  xcd[:])


def _scan_stage(nc, tc, dram, dbg, wp, env):
    xdbl_bf = env["xdbl_bf"]
    yg_bf = env["yg_bf"]
    xc_own = env["xc_own"]
    gz = env["gz"]
    bc_scr = env["bc_scr"]
    w_dtT = env["w_dtT"]
    A_col = env["A_col"]
    b_dt = env["b_dt"]
    dp_diag = env["dp_diag"]
    idn = env["idn"]
    dtT_bf = xdbl_bf[0:DT_RANK, :]

    with tc.tile_pool(name="scanp", bufs=1) as sp, \
         tc.tile_pool(name="ps_mm4", bufs=2, space="PSUM") as ps4, \
         tc.tile_pool(name="ps_y", bufs=1, space="PSUM") as psy:
        for r in range(NDT):
            # ---- mdelta = -softplus(dt @ W_dt.T + b_dt) = ln(sigmoid(-x))
            # (host negates b_dt, A, and the B rows of W_x to absorb the
            # sign, keeping the whole delta path on ScalarE) ----
            sig = sp.tile([128, L], F32, name="sig", tag="sig")
            for n in range(NCH):
                ps = ps4.tile([128, CW], F32, name="ps4t", tag="ps4t")
                nc.tensor.matmul(ps[:], w_dtT[:, r * 128:(r + 1) * 128],
                                 dtT_bf[:, n * CW:(n + 1) * CW],
                                 start=True, stop=True)
                nc.scalar.activation(sig[:, n * CW:(n + 1) * CW], ps[:],
                                     AF.Sigmoid, bias=b_dt[r][:],
                                     scale=-1.0)
            mdelta = sp.tile([128, L], BF16, name="mdelta", tag="mdelta",
                             bufs=2)
            nc.scalar.activation(mdelta[:], sig[:], AF.Ln)
            if dbg == 1:
                dd = sp.tile([128, L], F32, name="dd", tag="dd")
                nc.vector.tensor_copy(dd[:], mdelta[:])
                nc.sync.dma_start(
                    dram["delta_dbg"][r * 128:(r + 1) * 128, :], dd[:])

            # ---- du = mdelta * xc (sign fixed by negated B rows) ----
            du = sp.tile([128, L], BF16, name="du", tag="du", bufs=2)
            nc.vector.tensor_tensor(du[:], mdelta[:], xc_own[r][:], OP.mult)

            yp = [psy.tile([128, CW], F32, name=f"yp{n}", tag=f"yp{n}")
                  for n in range(NCH)]

            for s in range(D_STATE):
                # dA plane straight from ScalarE
                if s < N_S_F32:
                    dA = sp.tile([128, L], F32, name="ef", tag="ef", bufs=2)
                else:
                    dA = sp.tile([128, L], BF16, name="eb", tag="eb",
                                 bufs=2)
                nc.scalar.activation(dA[:], mdelta[:], AF.Exp, bias=0.0,
                                     scale=A_col[r][:, s:s + 1])
                b_rep = sp.tile([128, L], BF16, name="b_rep", tag="b_rep",
                                bufs=3)
                nc.sync.dma_start(
                    b_rep[:], bc_scr[s:s + 1, :].broadcast_to((128, L)))
                c_rep = sp.tile([128, L], BF16, name="c_rep", tag="c_rep",
                                bufs=3)
                nc.sync.dma_start(
                    c_rep[:], bc_scr[D_STATE + s:D_STATE + s + 1, :]
                    .broadcast_to((128, L)))
                dbu = sp.tile([128, L], BF16, name="dbu", tag="dbu",
                              bufs=3)
                nc.vector.tensor_tensor(dbu[:], du[:], b_rep[:], OP.mult)
                h = sp.tile([128, L], BF16, name="h", tag="h", bufs=3)
                nc.vector.tensor_tensor_scan(h[:], dA[:], dbu[:], 0.0,
                                             OP.mult, OP.add)
                ws = sp.tile([128, L], BF16, name="ws", tag="ws", bufs=2)
                nc.vector.tensor_tensor(ws[:], h[:], c_rep[:], OP.mult)
                for n in range(NCH):
                    nc.tensor.matmul(yp[n][:], idn[:],
                                     ws[:, n * CW:(n + 1) * CW],
                                     start=(s == 0), stop=False)
            # skip term
            for n in range(NCH):
                nc.tensor.matmul(yp[n][:], dp_diag[r][:],
                                 xc_own[r][:, n * CW:(n + 1) * CW],
                                 start=False, stop=True)
            # gate with silu(z)
            for n in range(NCH):
                nc.vector.tensor_tensor(yg_bf[r][:, n * CW:(n + 1) * CW],
                                        yp[n][:],
                                        gz[r][:, n * CW:(n + 1) * CW],
                                        OP.mult)


# ======================= host side =======================

def _prep_core_inputs(inputs, b, rev, h):
    hs = np.asarray(inputs["hidden_states"])
    W_in = np.asarray(inputs["W_in"])
    conv_w = np.asarray(inputs["conv_w"])[:, 0, :]
    conv_b = np.asarray(inputs["conv_b"])
    W_x = np.asarray(inputs["W_x"])
    W_dt = np.asarray(inputs["W_dt"])
    b_dt = np.asarray(inputs["b_dt"])
    A = -np.exp(np.asarray(inputs["A_log"], np.float64)).astype(np.float32)
    Dp = np.asarray(inputs["Dp"])
    W_out = np.asarray(inputs["W_out"])

    lo, hi = h * HALF, (h + 1) * HALF
    perm = np.r_[lo:hi, (0 if h else HALF):(HALF if h else D_INNER)]

    u = hs[b]
    if rev:
        u = u[::-1]
    uT = np.ascontiguousarray(u.T).astype(BF_NP)

    W_in_x = W_in[0:D_INNER][perm]
    W_in_z = W_in[D_INNER + lo:D_INNER + hi]
    conv_wp = conv_w[perm]
    conv_bp = conv_b[perm].reshape(-1, 1).astype(np.float32)
    W_xp = W_x[:, perm]
    W_xpad = np.zeros((NXP, W_xp.shape[1]), W_xp.dtype)
    W_xpad[0:DT_RANK] = W_xp[0:DT_RANK]
    # B rows negated: device uses mdelta = -delta, so du = -delta*xc and
    # (-B)*du = delta*xc*B. C rows (80:96) keep their sign.
    W_xpad[64:80] = -W_xp[DT_RANK:DT_RANK + D_STATE]
    W_xpad[80:96] = W_xp[DT_RANK + D_STATE:NXD]

    conv_diag = np.zeros((NDT_FULL * D_CONV * 128, 128), np.float32)
    idx = np.arange(128)
    for r in range(NDT_FULL):
        for j in range(D_CONV):
            base = (r * D_CONV + j) * 128
            conv_diag[base + idx, idx] = conv_wp[r * 128:(r + 1) * 128, j]

    dp_diag = np.zeros((NDT * 128, 128), np.float32)
    for r in range(NDT):
        dp_diag[r * 128 + idx, idx] = Dp[lo + r * 128: lo + (r + 1) * 128]

    return {
        "uT": uT,
        "w_in_xT": np.ascontiguousarray(W_in_x.T).astype(BF_NP),
        "w_in_zT": np.ascontiguousarray(W_in_z.T).astype(BF_NP),
        "conv_diag": conv_diag.astype(BF_NP),
        "conv_b": conv_bp,
        "w_xT": np.ascontiguousarray(W_xpad.T).astype(BF_NP),
        "w_dtT": np.ascontiguousarray(W_dt[lo:hi].T).astype(BF_NP),
        "b_dt": -b_dt[lo:hi].reshape(-1, 1).astype(np.float32),
        "A_half": -A[lo:hi].astype(np.float32),
        "dp_diag": dp_diag.astype(BF_NP),
        "idn": np.eye(128, dtype=np.float32).astype(BF_NP),
        "w_outT": np.ascontiguousarray(W_out[:, lo:hi].T).astype(BF_NP),
    }


_CACHE = {}


def kernel(**inputs):
    if "prog" not in _CACHE:
        _CACHE["prog"] = build_program(0)
    nc = _CACHE["prog"]

    in_maps = []
    for c in range(8):
        b, rev, h = c >> 2, (c >> 1) & 1, c & 1
        in_maps.append(_prep_core_inputs(inputs, b, rev, h))
    res = run_bass_kernel_spmd(nc, in_maps, list(range(8)))

    out = np.zeros((BATCH, L, D_MODEL), np.float32)
    for c in range(8):
        b, rev, h = c >> 2, (c >> 1) & 1, c & 1
        part = res.results[c]["out_part"].T
        if rev:
            part = part[::-1]
        out[b] += part
    return out
